# revision 1
# baseline (speedup 1.0000x reference)
"""Trainium2 Bass kernel for nn_Attention_6322191859738.

Reference computation (b=1, c=64, n=16^3=4096, heads=4, dim_head=32):
    qkv = w_qkv @ x            # 1x1 conv == channel matmul
    per head: sim = (q*scale)^T k ; attn = softmax(sim, axis=keys)
              out = attn @ v^T
    y = w_out @ out + b_out

Sharding: 8 cores, each core owns a contiguous chunk of 512 query
positions (n-dim) and computes ALL heads for that chunk.  k/v are
computed redundantly on every core (cheap).  Output is a clean concat
over the n axis -- no collectives, no host reduction.

Per-core layout (all on one NeuronCore, fp32 data, float32r matmuls):
    z_h = (Wq_h^T Wk_h scale).T @ xq          (host-folded q/k projections)
    sim_T[j, i] = x_blk.T @ z_h               (PE, keys j on partitions)
    exp_T = exp(sim_T)                        (ACT, PSUM->SBUF, groups of 3
                                               PSUM banks, double buffered)
    out_aug[(d|1), i] = sum_j vaug[j,(d|1)] exp_T[j, i]   (PE, PSUM accum)
      where vaug = transposed v with a ones column -> row 32 accumulates
      the softmax denominator for free
    out[d, i] = out_aug[d, i] * recip(out_aug[32, i])     (DVE + GpSimd)
    y = w_out^T.T @ out + b                   (PE + DVE)

The attn@v matmuls run one exp-group behind the sim matmuls so head
boundaries never stall PE on the last exp; transposed-v projections ride
the spare PSUM bank during head 0; dummy matmuls during the input DMA
release the PE HAM clock gate before real work arrives.
"""

import os
import sys

import numpy as np

HEADS = 4
D = 32          # dim_head
C = 64          # channels
N = 4096        # spatial positions (16^3)
NCORES = 8
NQ = N // NCORES  # queries per core = 512
HID = HEADS * D   # 128
JT = N // 128     # 32 j-tiles of 128 keys
GROUP = 3         # j-tiles per exp group (3 PSUM banks, x2 buffers)

_CACHE = {}


def _ensure_paths():
    for p in ("/opt/trn_rl_repo",):
        if p not in sys.path and os.path.isdir(p):
            sys.path.insert(0, p)


def _build(mm_dtype_name="float32r", reps=1, mm2_wide=False, group=GROUP,
           psim_bufs=2):
    """Build + compile the per-core Bass module (cached).

    reps > 1 unrolls the whole computation `reps` times inside one NEFF --
    used only for timing (wall-clock slope removes the RPC overhead).
    mm2_wide: use M=128 stationary slices for the attn@v matmul so MM1
    and MM2 share both K and M (no PE shape switches at all); rows 33..127
    of the accumulator are junk and ignored.
    group/psim_bufs: j-tiles per exp group x PSUM buffering
    (group*psim_bufs + 2 oa banks must be <= 8)."""
    key = (mm_dtype_name, reps, mm2_wide, group, psim_bufs)
    if key in _CACHE:
        return _CACHE[key]
    _ensure_paths()
    import concourse.bass as bass
    import concourse.tile as tile
    from concourse import bacc, mybir

    f32 = mybir.dt.float32
    mmdt = getattr(mybir.dt, mm_dtype_name)

    nc = bacc.Bacc(
        "TRN2",
        target_bir_lowering=False,
        debug=False,
        enable_asserts=False,
    )

    # packed inputs, two tensors:
    #   bigA [128, 833] = [at4 (512) | wv2 (256) | wo (64) | one (1)]
    #     -- rows that must be exact (zero-padded where needed)
    #   bigB [64, 4609] = [xq (512) | x (4096) | b (1)]
    #     -- DMA'd into rows 0:64 of a [128, *] tile; rows 64:127 are
    #        filled with ones by GpSimd in parallel (they only ever
    #        multiply exactly-zero z/wv2 rows, so any finite value works)
    # Every matmul operand is padded to K=128: the PE pays ~460ns each
    # time consecutive matmuls change the contraction dim (measured), so
    # the whole kernel uses K=128 everywhere.
    FA = 512 + 256 + 64 + 1
    FB = NQ + N + 1
    bigA_d = nc.dram_tensor("bigA", [128, FA], mmdt, kind="ExternalInput").ap()
    bigB_d = nc.dram_tensor("bigB", [C, FB], mmdt, kind="ExternalInput").ap()
    y_d = nc.dram_tensor("y", [C, NQ], f32, kind="ExternalOutput").ap()

    Exp = mybir.ActivationFunctionType.Exp

    with tile.TileContext(nc) as tc:
        with (
            tc.tile_pool(name="consts", bufs=1) as consts,
            tc.tile_pool(name="persist", bufs=1) as persist,
            tc.tile_pool(name="exp", bufs=4) as exp_pool,
            tc.tile_pool(name="small", bufs=2) as small,
        ):
            # ---- load inputs: 3 DMA instructions; GpSimd fills the
            # junk rows of the x/xq tile while the DMAs stream ----
            bigA = consts.tile([128, FA], mmdt)
            nc.sync.dma_start(bigA[:, 0:65], bigA_d[:, 0:65])
            nc.sync.dma_start(bigA[:, 65:FA], bigA_d[:, 65:FA])
            bigB = consts.tile([128, FB], mmdt)
            nc.sync.dma_start(bigB[0:C, 0:512], bigB_d[:, 0:512])
            nc.sync.dma_start(bigB[0:C, 512:1536], bigB_d[:, 512:1536])
            nc.sync.dma_start(bigB[0:C, 1536:3584], bigB_d[:, 1536:3584])
            nc.sync.dma_start(bigB[0:C, 3584:FB], bigB_d[:, 3584:FB])
            wo_sb = bigA[:, 0:64]
            one_sb = bigA[:, 64:65]
            at_sb = bigA[:, 65:577]
            wv2_sb = bigA[:, 577:833]
            xq_sb = bigB[:, 0:NQ]
            x_sb = bigB[:, NQ:NQ + N]
            b_sb = bigB[0:C, NQ + N:NQ + N + 1].bitcast(f32)
            nc.vector.tensor_copy(          # fast: unblocks z/xq
                bigB[C:128, 0:1536],
                one_sb[0:C, 0:1].broadcast_to((C, 1536)))
            for c0, c1 in ((1536, 3072), (3072, FB)):
                nc.gpsimd.tensor_copy(
                    bigB[C:128, c0:c1],
                    one_sb[0:C, 0:1].broadcast_to((C, c1 - c0)))

            rep_bodies(nc, tc, reps, locals(), mm2_wide, group, psim_bufs)

    nc.compile()
    _CACHE[key] = nc
    return nc


def rep_bodies(nc, tc, reps, env, mm2_wide=False, group=GROUP, psim_bufs=2):
    for _rep in range(reps):
        _emit_body(nc, tc, env, mm2_wide, warmup=(_rep == 0),
                   group=group, psim_bufs=psim_bufs)


def _emit_body(nc, tc, env, mm2_wide=False, warmup=True,
               group=GROUP, psim_bufs=2):
    from concourse import mybir
    f32 = mybir.dt.float32
    mmdt = env["mmdt"]
    Exp = env["Exp"]
    consts = env["consts"]; persist = env["persist"]
    exp_pool = env["exp_pool"]; small = env["small"]
    x_sb = env["x_sb"]; xq_sb = env["xq_sb"]; at_sb = env["at_sb"]
    wv2_sb = env["wv2_sb"]; wo_sb = env["wo_sb"]
    b_sb = env["b_sb"]; one_sb = env["one_sb"]; y_d = env["y_d"]

    # persistent SBUF tensors.  z_h = AT_h.T @ xq  [64, NQ]: the folded
    # q/k projection product, the moving operand of every MM1.
    z = [persist.tile([128, NQ], mmdt, name=f"z{h}", tag=f"z{h}")
         for h in range(HEADS)]
    onorm = persist.tile([HID, NQ], mmdt)   # normalized attn out, [hd, i]
    # vaug: per j-tile [j, (h, d|1)] transposed v with a ones column per
    # head -- one big tile, viewed [128, jt, h, d+1]
    vaug = persist.tile([128, JT * HEADS * (D + 1) + 95], mmdt, name="vaug")
    vaug4 = vaug[:, 0:JT * HEADS * (D + 1)].rearrange("p (j h w) -> p j h w", j=JT, h=HEADS)

    # warm the ACT exp table set early so the ~1.3us load overlaps the
    # projection prologue
    wtmp = small.tile([1, 1], f32, tag="wtmp")
    nc.scalar.activation(wtmp[:], one_sb[0:1, :], Exp)

    # all ones columns in one strided write
    nc.vector.tensor_copy(
        vaug4[:, :, :, D:D + 1],
        one_sb[:, None, None, :].broadcast_to((128, JT, HEADS, 1)))
    # init the tail padding read (as junk) by wide MM2 stationary slices
    nc.vector.tensor_copy(
        vaug[:, JT * HEADS * (D + 1):JT * HEADS * (D + 1) + 95],
        one_sb[:, 0:1].broadcast_to((128, 95)))

    groups = [list(range(g, min(g + group, JT))) for g in range(0, JT, group)]

    # PSUM budget (8 banks): psim 2x3 + oa 2x1.  All projections ride
    # PSUM pool slots (PE executes its queue in order, so a single-bank
    # projection rotation would head-of-line-block the whole engine).
    with (
        tc.tile_pool(name="psum_sim", bufs=psim_bufs, space="PSUM") as psim,
        tc.tile_pool(name="psum_oa", bufs=2, space="PSUM") as poa,
    ):
        # ---- PE warmup: dense dummy matmuls on big128 while the input
        # DMAs stream, so the HAM clock gate is released (2.4 GHz) by the
        # time real work arrives ----
        if warmup:
            wup = poa.tile([128, 512], f32, tag="oa")
            for i in range(12):
                nc.tensor.matmul(wup[0:C, (i % 8) * 64:(i % 8 + 1) * 64],
                                 wo_sb[:], wo_sb[:, 0:64],
                                 start=True, stop=True)
            wscrap = small.tile([1, 1], f32, tag="wtmp")
            nc.vector.tensor_copy(wscrap[:], wup[0:1, 0:1])

        # ---- z_h = AT_h.T @ xq through psim slots (K=128, M=128;
        # AT zero-padding makes rows 64..127 of z exactly zero).  Copies
        # alternate DVE/ACT so the prologue chain isn't DVE-serial; only
        # the first pair is emitted up front (head 0 needs just z0) ----
        def z_pair(b):
            # pair 0 (prologue, ACT idle): copies split DVE/ACT to halve
            # the serial chain.  pair 1 (mid-stream, ACT is the
            # bottleneck): both copies on DVE.
            zp = psim.tile([128, 512 * group], f32, tag="sp", name=f"zp{b}")
            for t in range(2):
                h = 2 * b + t
                sl = zp[:, t * 512:(t + 1) * 512]
                nc.tensor.matmul(sl, at_sb[:, h * 128:(h + 1) * 128],
                                 xq_sb[:], start=True, stop=True)
                if b == 0 and t == 1:
                    nc.scalar.copy(z[h][:], sl)
                else:
                    nc.vector.tensor_copy(z[h][:], sl)

        z_pair(0)

        # ---- transposed-v projections: 2 j-tiles per PSUM-bank turn,
        # riding the spare oa slot so they never steal an ACT beat ----
        # vt[j, hd] = x_blk.T @ wv; wv2 = [wvT|wvT] pads the moving dim
        # to 256 so float32r runs at full rate.
        def vt_turn(jts):
            vp = poa.tile([128, 512], f32, tag="oa")
            for t, jt in enumerate(jts):
                nc.tensor.matmul(vp[:, t * 256:(t + 1) * 256],
                                 x_sb[:, jt * 128:(jt + 1) * 128],
                                 wv2_sb[:], start=True, stop=True)
            nb = len(jts)
            vsrc = vp[:, 0:nb * 256].rearrange(
                "p (j hw) -> p j hw", j=nb)[:, :, 0:HID].rearrange(
                "p j (h w) -> p j h w", h=HEADS)
            nc.vector.tensor_copy(vaug4[:, jts[0]:jts[0] + nb, :, 0:D], vsrc)

        vt_turn([0, 1])
        vt_turn([2, 3])
        next_vt = 4
        zb_emitted = False

        def normalize(h, oa):
            # out[d,i] = oa[d,i] / oa[D,i], in column halves so the
            # recip -> broadcast -> mul chain pipelines across DVE/GpSimd
            for half in range(2):
                cs = slice(half * (NQ // 2), (half + 1) * (NQ // 2))
                rc = small.tile([1, NQ // 2], f32, tag="rc")
                nc.vector.reciprocal(rc[:], oa[D:D + 1, cs])
                bc = small.tile([D, NQ // 2], f32, tag="bc")
                nc.gpsimd.partition_broadcast(bc[:], rc[0:1, :])
                nc.vector.tensor_mul(onorm[h * D:(h + 1) * D, cs],
                                     oa[0:D, cs], bc[:])

        # ---- main attention loop: MM2s run one group behind MM1/exp so
        # head boundaries never stall PE on the last exp ----
        oa_tiles = {}
        pending = None          # (h, tiles, ex, is_last_group)
        flat = [(h, tiles) for h in range(HEADS) for tiles in groups]

        def flush(pend):
            h, tiles, ex, last = pend
            if h not in oa_tiles:
                oa_tiles[h] = poa.tile([128 if mm2_wide else D + 1, NQ], f32, name=f"oa{h}", tag="oa")
            oa = oa_tiles[h]
            W = D + 1
            MW = 128 if mm2_wide else D + 1
            for t, jt in enumerate(tiles):
                base = (jt * HEADS + h) * W
                nc.tensor.matmul(
                    oa[:],
                    vaug[:, base:base + MW],
                    ex[:, t * 512:(t + 1) * 512],
                    start=(jt == 0), stop=(jt == JT - 1))
            if last:
                normalize(h, oa)

        for h, tiles in flat:
            w = 512 * len(tiles)
            sp = psim.tile([128, 512 * group], f32, tag="sp")
            for t, jt in enumerate(tiles):
                nc.tensor.matmul(
                    sp[:, t * 512:(t + 1) * 512],
                    x_sb[:, jt * 128:(jt + 1) * 128],
                    z[h][:], start=True, stop=True)
            ex = exp_pool.tile([128, 512 * group], mmdt, tag="ex")
            nc.scalar.activation(ex[:, 0:w], sp[:, 0:w], Exp)
            if pending is not None:
                flush(pending)
            pending = (h, tiles, ex, tiles[-1] == JT - 1)
            if h == 0:
                if not zb_emitted:
                    z_pair(1)
                    zb_emitted = True
                for _ in range(2):
                    if next_vt < JT:
                        vt_turn([next_vt, next_vt + 1])
                        next_vt += 2
        flush(pending)

        # ---- output projection + bias, in column halves ----
        yp = poa.tile([128, 512], f32, tag="oa")
        for half in range(2):
            cs = slice(half * (NQ // 2), (half + 1) * (NQ // 2))
            nc.tensor.matmul(yp[0:C, cs], wo_sb[:], onorm[:, cs],
                             start=True, stop=True)
            y_sb = small.tile([C, NQ // 2], f32, tag="ysb")
            nc.vector.tensor_scalar_add(y_sb[:], yp[0:C, cs], b_sb[:])
            nc.sync.dma_start(y_d[:, cs], y_sb[:])


def make_in_maps(x, w_qkv, w_out, b_out):
    """Host-side prep: per-core input dicts (all fp32, C-contiguous).

    Operands that feed matmul contraction rows 64..127 with *meaningful
    zeros* (at4, wv2) are zero-padded here; x/xq ship as 64 rows and the
    device fills their junk rows (which only multiply zeros) with ones.
    """
    scale = D ** -0.5
    x = np.asarray(x, np.float32)
    xf = np.ascontiguousarray(x.reshape(C, N))
    w_qkv = np.asarray(w_qkv, np.float64)
    wq = w_qkv[0:HID] * scale          # [hd, c]
    wk = w_qkv[HID:2 * HID]            # [hd, c]

    def pad128(a):
        out = np.zeros((128, a.shape[1]), np.float32)
        out[0:a.shape[0]] = a
        return out

    # at4[h]: [128, 128] = [[Wq_h^T Wk_h, 0], [0, 0]]
    at4 = np.zeros((128, 512), np.float32)
    for h in range(HEADS):
        at4[0:C, h * 128:h * 128 + C] = (
            wq[h * D:(h + 1) * D].T @ wk[h * D:(h + 1) * D])
    wvT = w_qkv[2 * HID:3 * HID].T.astype(np.float32)          # [C, HID]
    wv2 = pad128(np.concatenate([wvT, wvT], 1))                # [128, 256]
    woT = np.ascontiguousarray(np.asarray(w_out, np.float32).T)  # [128, C]
    one = np.ones((128, 1), np.float32)
    bigA = np.ascontiguousarray(
        np.concatenate([woT, one, at4, wv2], 1))               # [128, 833]
    b = np.asarray(b_out, np.float32).reshape(C, 1)
    in_maps = []
    for c in range(NCORES):
        bigB = np.concatenate([xf[:, c * NQ:(c + 1) * NQ], xf, b], 1)
        in_maps.append({"bigA": bigA, "bigB": np.ascontiguousarray(bigB)})
    return in_maps


def kernel(x, w_qkv, w_out, b_out, _trace=False):
    _ensure_paths()
    from concourse.bass_utils import run_bass_kernel_spmd

    nc = _build()
    in_maps = make_in_maps(x, w_qkv, w_out, b_out)
    res = run_bass_kernel_spmd(nc, in_maps, core_ids=list(range(NCORES)),
                               trace=_trace)
    y = np.empty((C, N), np.float32)
    for c in range(NCORES):
        y[:, c * NQ:(c + 1) * NQ] = res.results[c]["y"]
    out = y.reshape(1, C, 16, 16, 16)
    if _trace:
        return out, res
    return out

